# revision 29
# baseline (speedup 1.0000x reference)
"""Multi-head attention on 8 Trainium2 NeuronCores — fp16 pipeline.

Sharding: tensor-parallel over heads (2 heads/core), full batch on every
core; host sums the 8 partial outputs and adds b_o + b_v @ w_o.

All matmul operands fp16 (1 cycle/row; fp8 DoubleRow was measured to
give no gain at contraction=64 — the PE is output-row bound, not
contraction bound — while doubling LDWEIGHTS serialization).

Pipeline (per batch bi):
  A: xT[kt] [128, S] fp16 DMA'd in 1024-col chunks (2KB/partition
     lines keep DMA at full rate), kt-major per chunk, so batch-0
     projections start after 2MB instead of 4MB.
  B: QK projections (fp16 MM -> PSUM; DVE bias-add -> fp16),
     V natural via lhsT=xT tile.
  C: jc-OUTER (2 chunks of 1024 q), h inner; per kt: S^T MM,
     D-unit pop (every other slot; the final batch holds a 10-unit
     reserve for the tail), exp (ACT), AV MM (pexp[kt-2] stagger so the
     PE never waits on ACT) with a ones row accumulating r.
     r-path: r row copied out of PSUM, reshaped [1,qw]->[128,qw/128]
     via SBUF->SBUF DMA round-trip on the ACT hwdge queue (DVE
     reciprocal costs ~6c per free-dim element, so the fat layout is
     ~30x cheaper than the row), reciprocal, DMA back, GpSimd
     partition_broadcast to [128,qw], DVE in-place multiply of the raw
     attnT rows -> attnT holds softmax-normalized attention.
  D: per (tt, half): ONE K=128 matmul po = attnT[:,tile]^T @ w_o (both
     heads in the contraction; no per-head rescale needed since attnT
     is pre-normalized), PSUM->SBUF fp16 copy (DVE / ACT on the tail),
     one paired DMA per tt. D(bi)-jc0 units interleave into C(bi)-jc1;
     D(bi)-jc1 units into C(bi+1)-jc0; only b3-jc1 flushes at the end.
"""

import numpy as np

import concourse.bacc as bacc
import concourse.mybir as mybir
from concourse.tile import TileContext
from concourse import bass_utils

dt = mybir.dt
F32 = dt.float32
F16 = dt.float16
AF = mybir.ActivationFunctionType
ALU = mybir.AluOpType

B, S, D = 4, 2048, 1024
H, DH = 16, 64
NCORES = 8
HPC = H // NCORES          # heads per core = 2
DHC = HPC * DH             # 128 projection cols per core

_CACHE = {}


def build_nc(b=B, s=S):
    d = D
    n_tt = s // 128            # token tiles per batch
    n_kt = d // 128            # contraction tiles for projections
    qw = 1024 if s >= 1024 else s
    n_jc = s // qw             # q chunks per batch
    ntj = qw // 128            # token tiles per q chunk
    assert s % 512 == 0 and d == 1024
    assert n_jc == 2, "C-stage interleave schedule assumes two q chunks"

    nc = bacc.Bacc("TRN2", target_bir_lowering=False, debug=False)

    x_d = nc.dram_tensor("x", [b, s // qw, d, qw], F16,
                         kind="ExternalInput")
    wq_d = nc.dram_tensor("wq", [d, DHC], F16, kind="ExternalInput")
    wk_d = nc.dram_tensor("wk", [d, DHC], F16, kind="ExternalInput")
    wv_d = nc.dram_tensor("wv", [d, DHC], F16, kind="ExternalInput")
    bq_d = nc.dram_tensor("bq", [DHC, 1], F32, kind="ExternalInput")
    bk_d = nc.dram_tensor("bk", [DHC, 1], F32, kind="ExternalInput")
    wo_d = nc.dram_tensor("wo", [DHC, d], F16, kind="ExternalInput")
    out_d = nc.dram_tensor("out", [b, s, d], F16, kind="ExternalOutput")

    with TileContext(nc) as tc:
        with (
            tc.tile_pool(name="const", bufs=1) as cpool,
            tc.tile_pool(name="wpool", bufs=1) as wpool,
            tc.tile_pool(name="xt", bufs=2 * n_kt) as xt_pool,
            tc.tile_pool(name="qk", bufs=2) as qk_pool,
            tc.tile_pool(name="vt", bufs=3) as vt_pool,
            tc.tile_pool(name="at", bufs=2) as at_pool,
            tc.tile_pool(name="pexp", bufs=3) as pexp_pool,
            tc.tile_pool(name="rline", bufs=5) as rline_pool,
            tc.tile_pool(name="osb", bufs=4) as osb_pool,
            tc.tile_pool(name="ps", bufs=1, space="PSUM") as pp,
        ):
            # ---- constants & weights ----
            ones_col = cpool.tile([128, 32], F16, tag="ones_col")
            nc.vector.memset(ones_col[:, :], 1.0)

            bq = cpool.tile([DHC, 1], F32, tag="bq")
            bk = cpool.tile([DHC, 1], F32, tag="bk")
            nc.sync.dma_start(out=bq[:, :], in_=bq_d[:, :])
            nc.sync.dma_start(out=bk[:, :], in_=bk_d[:, :])

            w16 = {}
            for name, dram in (("q", wq_d), ("k", wk_d), ("v", wv_d)):
                wall = wpool.tile([128, n_kt * DHC], F16, tag=f"w_{name}",
                                  name=f"w_{name}")
                nc.sync.dma_start(
                    out=wall.rearrange("p (kt c) -> p kt c", kt=n_kt),
                    in_=dram[:, :].rearrange("(kt p) c -> p kt c", p=128),
                )
                for kt in range(n_kt):
                    w16[(name, kt)] = wall[:, kt * DHC:(kt + 1) * DHC]
            wo = cpool.tile([DHC, d], F16, tag="wo")
            nc.sync.dma_start(out=wo[:, :], in_=wo_d[:, :])

            # stage-D queue: (bi_out, attnT, tt, half); attnT is normalized
            d_queue = []
            osb_cur = [None]   # [128, 1024] tile shared by a tt's two halves

            def emit_d_unit(use_act=0):
                if not d_queue:
                    return
                bi_out, attnT_p, tt, half = d_queue.pop(0)
                cs = slice(half * 512, (half + 1) * 512)
                po = pp.tile([128, 512], F32, tag="poA", bufs=2, name="po")
                nc.tensor.matmul(
                    po[:, :], attnT_p[:, tt * 128:(tt + 1) * 128],
                    wo[:, cs], start=True, stop=True,
                )
                if half == 0:
                    osb_cur[0] = osb_pool.tile([128, 1024], F16, tag="osb",
                                               name="osb")
                osb = osb_cur[0]
                if use_act:
                    nc.scalar.copy(osb[:, cs], po[:, :])
                else:
                    nc.vector.tensor_copy(osb[:, cs], po[:, :])
                if half == 1:
                    nc.sync.dma_start(
                        out=out_d[bi_out, tt * 128:(tt + 1) * 128, :],
                        in_=osb[:, :],
                    )

            # r-path: DVE reciprocal cost scales with FREE size, so a
            # row-shaped [1, qw] reciprocal costs ~6.5us while [128, 8]
            # costs ~0.2us. Round-trip the row through a fat layout via
            # SBUF->SBUF DMAs (linearization order matches on both sides,
            # so reshape + elementwise + reshape-back is exact).
            r_pend = []       # deferred closures finishing r-paths

            def emit_norm_start(rline_t, attnT_t, jc, h):
                rfat = rline_pool.tile([128, qw // 128], F32, tag="rfat",
                                       bufs=3)
                nc.scalar.dma_start(
                    out=rfat[:, :],
                    in_=rline_t[0:1, :].rearrange(
                        "a (p c) -> a p c", p=128),
                )

                def finish():
                    nc.vector.reciprocal(rfat[:, :], rfat[:, :])
                    rrow = rline_pool.tile([1, qw], F32, tag="rrow", bufs=3)
                    nc.scalar.dma_start(
                        out=rrow[0:1, :].rearrange("a (p c) -> a p c", p=128),
                        in_=rfat[:, :],
                    )
                    rb = rline_pool.tile([128, qw], F32, tag="rb", bufs=3)
                    nc.gpsimd.partition_broadcast(rb[:, :], rrow[0:1, :])
                    hs = slice(h * 64, (h + 1) * 64)
                    qs = slice(jc * qw, (jc + 1) * qw)
                    nc.vector.tensor_tensor(
                        attnT_t[hs, qs], attnT_t[hs, qs], rb[hs, :], ALU.mult
                    )

                r_pend.append(finish)

            def drain_r_pend():
                while r_pend:
                    r_pend.pop(0)()

            prev_tail = None  # (bi, attnT, jc) of prev batch's last jc

            for bi in range(b):
                # ---- stage A: x^T DMA in 1024-col chunks (2KB/partition
                # lines keep the DMA at full rate), kt-major per chunk ----
                xT = [xt_pool.tile([128, s], F16, tag="xt", name=f"xT{kt}")
                      for kt in range(n_kt)]

                def emit_x_chunk(c4):
                    lo, hi = c4 * qw, (c4 + 1) * qw
                    for kt in range(n_kt):
                        nc.sync.dma_start(
                            out=xT[kt][:, lo:hi],
                            in_=x_d[bi, c4, kt * 128:(kt + 1) * 128, :],
                        )

                # ---- stage B: Q^T, K^T projections (fp16) ----
                qT = qk_pool.tile([DHC, s], F16, tag="qT")
                kT = qk_pool.tile([DHC, s], F16, tag="kT")
                n_c = s // qw
                for c in range(n_c):
                    emit_x_chunk(c)
                    for name, dst, bias in (("q", qT, bq), ("k", kT, bk)):
                        ppr = pp.tile([128, qw], F32, tag="st", bufs=2,
                                      name="ppr")
                        for kt in range(n_kt):
                            for j in range(qw // 512):
                                nc.tensor.matmul(
                                    ppr[:, j * 512:(j + 1) * 512],
                                    w16[(name, kt)],
                                    xT[kt][:, c * qw + j * 512:
                                            c * qw + (j + 1) * 512],
                                    start=(kt == 0),
                                    stop=(kt == n_kt - 1),
                                )
                        nc.vector.tensor_scalar_add(
                            dst[:, c * qw:(c + 1) * qw], ppr[:, :],
                            bias[:, 0:1],
                        )

                # V natural, interleaved-head layout [V_A |1| V_B |1] / 130
                vt = vt_pool.tile([128, n_tt * 130], F16, tag="vt")
                ones_dst = vt.rearrange("p (t two sv) -> p t two sv",
                                        two=2, sv=65)[:, :, :, 64]
                nc.vector.tensor_copy(ones_dst, ones_col[:, 0:2 * n_tt]
                                      .rearrange("p (t two) -> p t two", two=2))
                for tt in range(n_tt):
                    pv = pp.tile([128, 128], F32, tag="st", bufs=2, name="pv")
                    for kt in range(n_kt):
                        nc.tensor.matmul(
                            pv[:, :],
                            xT[kt][:, tt * 128:(tt + 1) * 128],
                            w16[("v", kt)],
                            start=(kt == 0),
                            stop=(kt == n_kt - 1),
                        )
                    vdst = vt.rearrange("p (t two sv) -> p t two sv",
                                        two=2, sv=65)[:, tt, :, 0:64]
                    nc.vector.tensor_copy(
                        vdst, pv.rearrange("p (two sv) -> p two sv", two=2)
                    )

                # finish pending r-paths (their DMAs have had time to
                # land), then queue the previous batch's last-jc D units
                drain_r_pend()
                if prev_tail is not None:
                    bi_p, attnT_p, jc_p = prev_tail
                    for tt in range(jc_p * ntj, (jc_p + 1) * ntj):
                        for half in range(2):
                            d_queue.append((bi_p, attnT_p, tt, half))
                    prev_tail = None

                # ---- stage C: attention, jc outer / h inner ----
                attnT = at_pool.tile([DHC, s], F16, tag="attnT")
                vtv = vt.rearrange("p (t two sv) -> p t two sv", two=2, sv=65)
                for jc in range(n_jc):
                    for h in range(HPC):
                        hs = slice(h * 64, (h + 1) * 64)
                        slot_base = jc * HPC * (n_tt + 1) + h * (n_tt + 1)
                        rline = rline_pool.tile([1, qw], F32, tag="rline")
                        qs = slice(jc * qw, (jc + 1) * qw)
                        av = pp.tile([65, qw], F32, tag="av", name="av")
                        pexps = {}
                        for kt in range(n_tt + 2):
                            if kt < n_tt:
                                st = pp.tile([128, qw], F32, tag="st",
                                             bufs=2, name="st")
                                for j in range(qw // 512):
                                    nc.tensor.matmul(
                                        st[:, j * 512:(j + 1) * 512],
                                        kT[hs, kt * 128:(kt + 1) * 128],
                                        qT[hs, jc * qw + j * 512:
                                           jc * qw + (j + 1) * 512],
                                        start=True, stop=True,
                                    )
                                if kt == 1:
                                    drain_r_pend()
                                # queue this batch's jc0 D units (attnT
                                # rows normalized at each chunk boundary)
                                if jc == 1 and h == 0 and kt == 3:
                                    for tt in range(ntj):
                                        for half in range(2):
                                            d_queue.append(
                                                (bi, attnT, tt, half))
                                # pop D units: every other slot early;
                                # every slot in the last chunk, except on
                                # the final batch where ~6 units are held
                                # back to cover the tail r-chain latency
                                last_chunk = (jc == n_jc - 1 and h == HPC - 1)
                                if last_chunk and bi == b - 1:
                                    if len(d_queue) > 10:
                                        emit_d_unit()
                                elif last_chunk \
                                        or (slot_base + kt) % 2 == 0:
                                    emit_d_unit()
                                pexp = pexp_pool.tile([128, qw], F16,
                                                      tag="pexp", name="pexp")
                                nc.scalar.activation(
                                    pexp[:, :], st[:, :], AF.Exp, scale=0.125
                                )
                                pexps[kt] = pexp
                            if kt >= 2:
                                px = pexps.pop(kt - 2)
                                for j in range(qw // 512):
                                    nc.tensor.matmul(
                                        av[:, j * 512:(j + 1) * 512],
                                        vtv[:, kt - 2, h, :],
                                        px[:, j * 512:(j + 1) * 512],
                                        start=(kt == 2),
                                        stop=(kt == n_tt + 1),
                                    )
                        nc.vector.tensor_copy(attnT[hs, qs], av[0:64, :])
                        nc.vector.tensor_copy(rline[0:1, :], av[64:65, :])
                        emit_norm_start(rline, attnT, jc, h)

                prev_tail = (bi, attnT, n_jc - 1)

            # ---- tail: flush the last batch's final-jc D units ----
            # drain the r-chain first (short DVE legs; its DMA triggers
            # must not queue behind ACT copies), then pop the reserved jc0
            # units starting on ACT so the PE stays fed while the chain's
            # DVE/GpSimd legs complete
            drain_r_pend()
            bi_p, attnT_p, jc_p = prev_tail
            for tt in range(jc_p * ntj, (jc_p + 1) * ntj):
                for half in range(2):
                    d_queue.append((bi_p, attnT_p, tt, half))
            eng = 1
            while d_queue:
                emit_d_unit(use_act=eng)
                eng = 1 - eng

    nc.compile()
    return nc


def _get_nc(b, s):
    key = (b, s)
    if key not in _CACHE:
        _CACHE[key] = build_nc(b, s)
    return _CACHE[key]


def make_in_maps(x, w_q, b_q, w_k, b_k, w_v, w_o):
    b, s, d_ = x.shape
    qw = 1024
    x16 = np.ascontiguousarray(
        np.asarray(x, dtype=np.float16).transpose(0, 2, 1)
        .reshape(b, d_, s // qw, qw).transpose(0, 2, 1, 3))
    wq16 = np.asarray(w_q, dtype=np.float16)
    wk16 = np.asarray(w_k, dtype=np.float16)
    wv16 = np.asarray(w_v, dtype=np.float16)
    wo16 = np.asarray(w_o, dtype=np.float16)
    in_maps = []
    for i in range(NCORES):
        cs = slice(i * DHC, (i + 1) * DHC)
        in_maps.append({
            "x": x16,
            "wq": np.ascontiguousarray(wq16[:, cs]),
            "wk": np.ascontiguousarray(wk16[:, cs]),
            "wv": np.ascontiguousarray(wv16[:, cs]),
            "bq": np.ascontiguousarray(b_q[cs, None], dtype=np.float32),
            "bk": np.ascontiguousarray(b_k[cs, None], dtype=np.float32),
            "wo": np.ascontiguousarray(wo16[cs, :]),
        })
    return in_maps


def kernel(x, w_q, b_q, w_k, b_k, w_v, b_v, w_o, b_o, _trace=False):
    x = np.asarray(x, dtype=np.float32)
    nc = _get_nc(x.shape[0], x.shape[1])
    in_maps = make_in_maps(x, w_q, b_q, w_k, b_k, w_v, w_o)
    kw = {}
    if _trace:
        import tempfile
        kw = dict(trace=True, trace_cores=list(range(NCORES)),
                  tmpdir=tempfile.mkdtemp(prefix="mha_trace_"))
    res = bass_utils.run_bass_kernel_spmd(
        nc, in_maps, core_ids=list(range(NCORES)), **kw
    )
    out = np.zeros(x.shape, dtype=np.float32)
    for i in range(NCORES):
        out += np.asarray(res.results[i]["out"], dtype=np.float32)
    out += np.asarray(b_o, dtype=np.float32)[None, None, :]
    out += (np.asarray(b_v, dtype=np.float32)
            @ np.asarray(w_o, dtype=np.float32))[None, None, :]
    if _trace:
        return out, res
    return out


# revision 31
# speedup vs baseline: 1.0130x; 1.0130x over previous
"""Multi-head attention on 8 Trainium2 NeuronCores — fp16 pipeline.

Sharding: tensor-parallel over heads (2 heads/core), full batch on every
core; host sums the 8 partial outputs and adds b_o + b_v @ w_o.

All matmul operands fp16 (1 cycle/row; fp8 DoubleRow was measured to
give no gain at contraction=64 — the PE is output-row bound, not
contraction bound — while doubling LDWEIGHTS serialization).

Pipeline (per batch bi):
  A: xT[kt] [128, S] fp16 DMA'd in 1024-col chunks (2KB/partition
     lines keep DMA at full rate), kt-major per chunk, so batch-0
     projections start after 2MB instead of 4MB.
  B: QK projections (fp16 MM -> PSUM; DVE bias-add -> fp16),
     V natural via lhsT=xT tile.
  C: jc-OUTER (2 chunks of 1024 q), h inner; per kt: S^T MM,
     D-unit pop (every other slot; the final batch holds a 10-unit
     reserve for the tail), exp (ACT), AV MM (pexp[kt-2] stagger so the
     PE never waits on ACT) with a ones row accumulating r.
     r-path: r row copied out of PSUM, reshaped [1,qw]->[128,qw/128]
     via SBUF->SBUF DMA round-trip on the ACT hwdge queue (DVE
     reciprocal costs ~6c per free-dim element, so the fat layout is
     ~30x cheaper than the row), reciprocal, DMA back, GpSimd
     partition_broadcast to [128,qw], DVE in-place multiply of the raw
     attnT rows -> attnT holds softmax-normalized attention.
  D: per (tt, half): ONE K=128 matmul po = attnT[:,tile]^T @ w_o (both
     heads in the contraction; no per-head rescale needed since attnT
     is pre-normalized), PSUM->SBUF fp16 copy (DVE / ACT on the tail),
     one paired DMA per tt. D(bi)-jc0 units interleave into C(bi)-jc1;
     D(bi)-jc1 units into C(bi+1)-jc0; only b3-jc1 flushes at the end.
"""

import numpy as np

import concourse.bacc as bacc
import concourse.mybir as mybir
from concourse.tile import TileContext
from concourse import bass_utils

dt = mybir.dt
F32 = dt.float32
F16 = dt.float16
AF = mybir.ActivationFunctionType
ALU = mybir.AluOpType

B, S, D = 4, 2048, 1024
H, DH = 16, 64
NCORES = 8
HPC = H // NCORES          # heads per core = 2
DHC = HPC * DH             # 128 projection cols per core

_CACHE = {}


def build_nc(b=B, s=S):
    d = D
    n_tt = s // 128            # token tiles per batch
    n_kt = d // 128            # contraction tiles for projections
    qw = 1024 if s >= 1024 else s
    n_jc = s // qw             # q chunks per batch
    ntj = qw // 128            # token tiles per q chunk
    assert s % 512 == 0 and d == 1024
    assert n_jc == 2, "C-stage interleave schedule assumes two q chunks"

    nc = bacc.Bacc("TRN2", target_bir_lowering=False, debug=False)

    x_d = nc.dram_tensor("x", [b, s // qw, d, qw], F16,
                         kind="ExternalInput")
    wq_d = nc.dram_tensor("wq", [128, d], F16, kind="ExternalInput")
    wk_d = nc.dram_tensor("wk", [128, d], F16, kind="ExternalInput")
    wv_d = nc.dram_tensor("wv", [128, d], F16, kind="ExternalInput")
    bq_d = nc.dram_tensor("bq", [DHC, 1], F32, kind="ExternalInput")
    bk_d = nc.dram_tensor("bk", [DHC, 1], F32, kind="ExternalInput")
    wo_d = nc.dram_tensor("wo", [DHC, d], F16, kind="ExternalInput")
    out_d = nc.dram_tensor("out", [b, s, d], F16, kind="ExternalOutput")

    with TileContext(nc) as tc:
        with (
            tc.tile_pool(name="const", bufs=1) as cpool,
            tc.tile_pool(name="wpool", bufs=1) as wpool,
            tc.tile_pool(name="xt", bufs=2 * n_kt) as xt_pool,
            tc.tile_pool(name="qk", bufs=2) as qk_pool,
            tc.tile_pool(name="vt", bufs=3) as vt_pool,
            tc.tile_pool(name="at", bufs=2) as at_pool,
            tc.tile_pool(name="pexp", bufs=3) as pexp_pool,
            tc.tile_pool(name="rline", bufs=5) as rline_pool,
            tc.tile_pool(name="osb", bufs=4) as osb_pool,
            tc.tile_pool(name="ps", bufs=1, space="PSUM") as pp,
        ):
            # ---- constants & weights ----
            ones_col = cpool.tile([128, 32], F16, tag="ones_col")
            nc.vector.memset(ones_col[:, :], 1.0)

            # batch-0 chunk-0 x DMAs go FIRST: each dma_start costs ~650ns
            # of serial issue time on the sync sequencer, and the first
            # matmul is gated on x arriving
            x0T = [xt_pool.tile([128, s], F16, tag="xt", name=f"xT{kt}")
                   for kt in range(n_kt)]
            for kt in range(n_kt):
                nc.sync.dma_start(
                    out=x0T[kt][:, 0:qw],
                    in_=x_d[0, 0, kt * 128:(kt + 1) * 128, :],
                )

            w16 = {}
            for name, dram in (("q", wq_d), ("k", wk_d), ("v", wv_d)):
                wall = wpool.tile([128, n_kt * DHC], F16, tag=f"w_{name}",
                                  name=f"w_{name}")
                nc.sync.dma_start(out=wall[:, :], in_=dram[:, :])
                for kt in range(n_kt):
                    w16[(name, kt)] = wall[:, kt * DHC:(kt + 1) * DHC]
            wo = cpool.tile([DHC, d], F16, tag="wo")
            nc.sync.dma_start(out=wo[:, :], in_=wo_d[:, :])
            bq = cpool.tile([DHC, 1], F32, tag="bq")
            bk = cpool.tile([DHC, 1], F32, tag="bk")
            nc.sync.dma_start(out=bq[:, :], in_=bq_d[:, :])
            nc.sync.dma_start(out=bk[:, :], in_=bk_d[:, :])

            # stage-D queue: (bi_out, attnT, tt, half); attnT is normalized
            d_queue = []
            osb_cur = [None]   # [128, 1024] tile shared by a tt's two halves

            def emit_d_unit(use_act=0):
                if not d_queue:
                    return
                bi_out, attnT_p, tt, half = d_queue.pop(0)
                cs = slice(half * 512, (half + 1) * 512)
                po = pp.tile([128, 512], F32, tag="poA", bufs=2, name="po")
                nc.tensor.matmul(
                    po[:, :], attnT_p[:, tt * 128:(tt + 1) * 128],
                    wo[:, cs], start=True, stop=True,
                )
                if half == 0:
                    osb_cur[0] = osb_pool.tile([128, 1024], F16, tag="osb",
                                               name="osb")
                osb = osb_cur[0]
                if use_act:
                    nc.scalar.copy(osb[:, cs], po[:, :])
                else:
                    nc.vector.tensor_copy(osb[:, cs], po[:, :])
                if half == 1:
                    nc.sync.dma_start(
                        out=out_d[bi_out, tt * 128:(tt + 1) * 128, :],
                        in_=osb[:, :],
                    )

            # r-path: DVE reciprocal cost scales with FREE size, so a
            # row-shaped [1, qw] reciprocal costs ~6.5us while [128, 8]
            # costs ~0.2us. Round-trip the row through a fat layout via
            # SBUF->SBUF DMAs (linearization order matches on both sides,
            # so reshape + elementwise + reshape-back is exact).
            r_pend = []       # deferred closures finishing r-paths

            def emit_norm_start(rline_t, attnT_t, jc, h):
                rfat = rline_pool.tile([128, qw // 128], F32, tag="rfat",
                                       bufs=3)
                nc.scalar.dma_start(
                    out=rfat[:, :],
                    in_=rline_t[0:1, :].rearrange(
                        "a (p c) -> a p c", p=128),
                )

                def finish():
                    nc.vector.reciprocal(rfat[:, :], rfat[:, :])
                    rrow = rline_pool.tile([1, qw], F32, tag="rrow", bufs=3)
                    nc.scalar.dma_start(
                        out=rrow[0:1, :].rearrange("a (p c) -> a p c", p=128),
                        in_=rfat[:, :],
                    )
                    rb = rline_pool.tile([128, qw], F32, tag="rb", bufs=3)
                    nc.gpsimd.partition_broadcast(rb[:, :], rrow[0:1, :])
                    hs = slice(h * 64, (h + 1) * 64)
                    qs = slice(jc * qw, (jc + 1) * qw)
                    nc.vector.tensor_tensor(
                        attnT_t[hs, qs], attnT_t[hs, qs], rb[hs, :], ALU.mult
                    )

                r_pend.append(finish)

            def drain_r_pend():
                while r_pend:
                    r_pend.pop(0)()

            prev_tail = None  # (bi, attnT, jc) of prev batch's last jc

            for bi in range(b):
                # ---- stage A: x^T DMA in 1024-col chunks (2KB/partition
                # lines keep the DMA at full rate), kt-major per chunk ----
                if bi == 0:
                    xT = x0T
                else:
                    xT = [xt_pool.tile([128, s], F16, tag="xt",
                                       name=f"xT{kt}")
                          for kt in range(n_kt)]

                def emit_x_chunk(c4):
                    if bi == 0 and c4 == 0:
                        return   # already issued before the weights
                    lo, hi = c4 * qw, (c4 + 1) * qw
                    for kt in range(n_kt):
                        nc.sync.dma_start(
                            out=xT[kt][:, lo:hi],
                            in_=x_d[bi, c4, kt * 128:(kt + 1) * 128, :],
                        )

                # ---- stage B: Q^T, K^T projections (fp16) ----
                qT = qk_pool.tile([DHC, s], F16, tag="qT")
                kT = qk_pool.tile([DHC, s], F16, tag="kT")
                n_c = s // qw
                for c in range(n_c):
                    emit_x_chunk(c)
                    for name, dst, bias in (("q", qT, bq), ("k", kT, bk)):
                        ppr = pp.tile([128, qw], F32, tag="st", bufs=2,
                                      name="ppr")
                        for kt in range(n_kt):
                            for j in range(qw // 512):
                                nc.tensor.matmul(
                                    ppr[:, j * 512:(j + 1) * 512],
                                    w16[(name, kt)],
                                    xT[kt][:, c * qw + j * 512:
                                            c * qw + (j + 1) * 512],
                                    start=(kt == 0),
                                    stop=(kt == n_kt - 1),
                                )
                        nc.vector.tensor_scalar_add(
                            dst[:, c * qw:(c + 1) * qw], ppr[:, :],
                            bias[:, 0:1],
                        )

                # V natural, interleaved-head layout [V_A |1| V_B |1] / 130
                vt = vt_pool.tile([128, n_tt * 130], F16, tag="vt")
                ones_dst = vt.rearrange("p (t two sv) -> p t two sv",
                                        two=2, sv=65)[:, :, :, 64]
                nc.vector.tensor_copy(ones_dst, ones_col[:, 0:2 * n_tt]
                                      .rearrange("p (t two) -> p t two", two=2))
                for tt in range(n_tt):
                    pv = pp.tile([128, 128], F32, tag="st", bufs=2, name="pv")
                    for kt in range(n_kt):
                        nc.tensor.matmul(
                            pv[:, :],
                            xT[kt][:, tt * 128:(tt + 1) * 128],
                            w16[("v", kt)],
                            start=(kt == 0),
                            stop=(kt == n_kt - 1),
                        )
                    vdst = vt.rearrange("p (t two sv) -> p t two sv",
                                        two=2, sv=65)[:, tt, :, 0:64]
                    nc.vector.tensor_copy(
                        vdst, pv.rearrange("p (two sv) -> p two sv", two=2)
                    )

                # finish pending r-paths (their DMAs have had time to
                # land), then queue the previous batch's last-jc D units
                drain_r_pend()
                if prev_tail is not None:
                    bi_p, attnT_p, jc_p = prev_tail
                    for tt in range(jc_p * ntj, (jc_p + 1) * ntj):
                        for half in range(2):
                            d_queue.append((bi_p, attnT_p, tt, half))
                    prev_tail = None

                # ---- stage C: attention, jc outer / h inner ----
                attnT = at_pool.tile([DHC, s], F16, tag="attnT")
                vtv = vt.rearrange("p (t two sv) -> p t two sv", two=2, sv=65)
                for jc in range(n_jc):
                    for h in range(HPC):
                        hs = slice(h * 64, (h + 1) * 64)
                        slot_base = jc * HPC * (n_tt + 1) + h * (n_tt + 1)
                        rline = rline_pool.tile([1, qw], F32, tag="rline")
                        qs = slice(jc * qw, (jc + 1) * qw)
                        av = pp.tile([65, qw], F32, tag="av", name="av")
                        pexps = {}
                        for kt in range(n_tt + 2):
                            if kt < n_tt:
                                st = pp.tile([128, qw], F32, tag="st",
                                             bufs=2, name="st")
                                for j in range(qw // 512):
                                    nc.tensor.matmul(
                                        st[:, j * 512:(j + 1) * 512],
                                        kT[hs, kt * 128:(kt + 1) * 128],
                                        qT[hs, jc * qw + j * 512:
                                           jc * qw + (j + 1) * 512],
                                        start=True, stop=True,
                                    )
                                if kt == 1:
                                    drain_r_pend()
                                # queue this batch's jc0 D units (attnT
                                # rows normalized at each chunk boundary)
                                if jc == 1 and h == 0 and kt == 3:
                                    for tt in range(ntj):
                                        for half in range(2):
                                            d_queue.append(
                                                (bi, attnT, tt, half))
                                # pop D units: every other slot early;
                                # every slot in the last chunk, except on
                                # the final batch where ~6 units are held
                                # back to cover the tail r-chain latency
                                last_chunk = (jc == n_jc - 1 and h == HPC - 1)
                                if last_chunk and bi == b - 1:
                                    if len(d_queue) > 10:
                                        emit_d_unit()
                                elif last_chunk \
                                        or (slot_base + kt) % 2 == 0:
                                    emit_d_unit()
                                pexp = pexp_pool.tile([128, qw], F16,
                                                      tag="pexp", name="pexp")
                                nc.scalar.activation(
                                    pexp[:, :], st[:, :], AF.Exp, scale=0.125
                                )
                                pexps[kt] = pexp
                            if kt >= 2:
                                px = pexps.pop(kt - 2)
                                for j in range(qw // 512):
                                    nc.tensor.matmul(
                                        av[:, j * 512:(j + 1) * 512],
                                        vtv[:, kt - 2, h, :],
                                        px[:, j * 512:(j + 1) * 512],
                                        start=(kt == 2),
                                        stop=(kt == n_tt + 1),
                                    )
                        nc.vector.tensor_copy(attnT[hs, qs], av[0:64, :])
                        nc.vector.tensor_copy(rline[0:1, :], av[64:65, :])
                        emit_norm_start(rline, attnT, jc, h)

                prev_tail = (bi, attnT, n_jc - 1)

            # ---- tail: flush the last batch's final-jc D units ----
            # drain the r-chain first (short DVE legs; its DMA triggers
            # must not queue behind ACT copies), then pop the reserved jc0
            # units starting on ACT so the PE stays fed while the chain's
            # DVE/GpSimd legs complete
            drain_r_pend()
            bi_p, attnT_p, jc_p = prev_tail
            for tt in range(jc_p * ntj, (jc_p + 1) * ntj):
                for half in range(2):
                    d_queue.append((bi_p, attnT_p, tt, half))
            eng = 1
            while d_queue:
                emit_d_unit(use_act=eng)
                eng = 1 - eng

    nc.compile()
    return nc


def _get_nc(b, s):
    key = (b, s)
    if key not in _CACHE:
        _CACHE[key] = build_nc(b, s)
    return _CACHE[key]


def make_in_maps(x, w_q, b_q, w_k, b_k, w_v, w_o):
    b, s, d_ = x.shape
    qw = 1024
    x16 = np.ascontiguousarray(
        np.asarray(x, dtype=np.float16).transpose(0, 2, 1)
        .reshape(b, d_, s // qw, qw).transpose(0, 2, 1, 3))
    wq16 = np.asarray(w_q, dtype=np.float16)
    wk16 = np.asarray(w_k, dtype=np.float16)
    wv16 = np.asarray(w_v, dtype=np.float16)
    wo16 = np.asarray(w_o, dtype=np.float16)

    def pack_w(w):
        # [1024, 128] core slice -> SBUF layout [128, 8*128]:
        # out[p, kt*128 + c] = w[kt*128 + p, c]
        return np.ascontiguousarray(
            w.reshape(8, 128, DHC).transpose(1, 0, 2).reshape(128, 8 * DHC))

    in_maps = []
    for i in range(NCORES):
        cs = slice(i * DHC, (i + 1) * DHC)
        in_maps.append({
            "x": x16,
            "wq": pack_w(wq16[:, cs]),
            "wk": pack_w(wk16[:, cs]),
            "wv": pack_w(wv16[:, cs]),
            "bq": np.ascontiguousarray(b_q[cs, None], dtype=np.float32),
            "bk": np.ascontiguousarray(b_k[cs, None], dtype=np.float32),
            "wo": np.ascontiguousarray(wo16[cs, :]),
        })
    return in_maps


def kernel(x, w_q, b_q, w_k, b_k, w_v, b_v, w_o, b_o, _trace=False):
    x = np.asarray(x, dtype=np.float32)
    nc = _get_nc(x.shape[0], x.shape[1])
    in_maps = make_in_maps(x, w_q, b_q, w_k, b_k, w_v, w_o)
    kw = {}
    if _trace:
        import tempfile
        kw = dict(trace=True, trace_cores=list(range(NCORES)),
                  tmpdir=tempfile.mkdtemp(prefix="mha_trace_"))
    res = bass_utils.run_bass_kernel_spmd(
        nc, in_maps, core_ids=list(range(NCORES)), **kw
    )
    out = np.zeros(x.shape, dtype=np.float32)
    for i in range(NCORES):
        out += np.asarray(res.results[i]["out"], dtype=np.float32)
    out += np.asarray(b_o, dtype=np.float32)[None, None, :]
    out += (np.asarray(b_v, dtype=np.float32)
            @ np.asarray(w_o, dtype=np.float32))[None, None, :]
    if _trace:
        return out, res
    return out
